# revision 43
# baseline (speedup 1.0000x reference)
"""Deformable conv block (B=8, C=64, H=W=128, K=3) on Trainium2.

Data-parallel over batch: one sample per core invocation.

Design (v4):
- Host pre-packs 5 x-shifted transposed tiles xall[x, s, (c, y+pad)] with
  zeros baked at image borders (no on-device transposes of x, no edge
  masks needed), plus the offset-conv operand xpad[65,(y,x)+pad] with a
  ones-row for the bias.
- Offset conv: per y-quarter round of 8x512 chunks, 10 PE matmuls/chunk
  (4 kx blocks packed in PSUM partitions + kx=4 separately), staged to
  SBUF, then 5 DMAs (1 copy + 4 accumulate) do the kx-shift reduction;
  clamp; xbar-transpose the quarter into offt[x,(y,d)]. Quarter rounds
  let the downstream mask/sampling pipeline start after round 0.
- Bilinear masks (per quarter): tensor_scalar (4x DVE mode) +
  tensor_tensor (2x) chain into gg[x,(t,ry,rx,y)].
- Sampling (per quarter, tap): DVE computes the 9 shift products
  (tensor_tensor, 2x mode); PE sums them via identity matmuls into PSUM
  f32; the PSUM->SBUF Act copy doubles as the (c,y)->(y,c) reorder; xbar
  transpose to [c,(y,x)]; per-tap weight matmuls accumulate all 9 taps
  in a second PSUM tile; result DMAs out via an SBUF f32 stage.
"""
import numpy as np
import ml_dtypes
from contextlib import ExitStack

import concourse.bass as bass
from concourse import bacc
import concourse.tile as tile
from concourse import mybir
from concourse.bass_utils import run_bass_kernel_spmd

bf16 = mybir.dt.bfloat16
f32 = mybir.dt.float32
Alu = mybir.AluOpType


def mkap(base_ap, extra_off, free_dims):
    """AP over base_ap's tensor: keep its partition dim, custom free dims."""
    p = list(base_ap.ap[0])
    return bass.AP(base_ap.tensor, base_ap.offset + extra_off, [p] + free_dims)


B, C, H, W = 8, 64, 128, 128
HW = H * W
NT = 9
YP = 132            # per-channel padded y extent (2 + 128 + 2)
XTF = C * YP        # 8448, one shift-tile's free size
QF = 4096           # quarter free size (32 y-rows x 128 x)


DEBUG_DUMPS = False


def _build():
    nc = bacc.Bacc()
    xall = nc.dram_tensor("xall", [128, 5 * XTF], bf16, kind="ExternalInput")
    xpad2 = nc.dram_tensor("xpad2", [128, 17408], bf16, kind="ExternalInput")
    wA2 = nc.dram_tensor("wA2", [128, 3 * 128], bf16, kind="ExternalInput")
    wB2 = nc.dram_tensor("wB2", [128, 3 * 18], bf16, kind="ExternalInput")
    offbx = nc.dram_tensor("offbx", [128, 18], bf16, kind="ExternalInput")
    wM = nc.dram_tensor("wM", [64, NT * 64], bf16, kind="ExternalInput")
    wM2 = nc.dram_tensor("wM2", [128, 4 * 64], bf16, kind="ExternalInput")
    id128 = nc.dram_tensor("id128", [128, 128], bf16, kind="ExternalInput")
    out = nc.dram_tensor("out", [C, HW], f32, kind="ExternalOutput")
    if DEBUG_DUMPS:
        dbg_off = nc.dram_tensor("dbg_off", [32, HW], bf16, kind="ExternalOutput")

    with tile.TileContext(nc, pool_alloc_mode="queue") as tc, ExitStack() as ctx:
        pw = ctx.enter_context(tc.tile_pool(name="pw", bufs=1))
        wAt = pw.tile([128, 3 * 128], bf16, name="wAt")
        nc.sync.dma_start(wAt[:], wA2[:])
        wBt = pw.tile([128, 3 * 18], bf16, name="wBt")
        nc.sync.dma_start(wBt[:], wB2[:])
        offbt = pw.tile([128, 18], bf16, name="offbt")
        nc.sync.dma_start(offbt[:], offbx[:])
        wMt = pw.tile([64, NT * 64], bf16, name="wMt")
        nc.sync.dma_start(wMt[:], wM[:])
        wM2t = pw.tile([128, 4 * 64], bf16, name="wM2t")
        nc.sync.dma_start(wM2t[:], wM2[:])
        idt = pw.tile([128, 128], bf16, name="idt")
        nc.sync.dma_start(idt[:], id128[:])

        # long-lived tensors
        pxa = ctx.enter_context(tc.tile_pool(name="pxa", bufs=1))
        xat = pxa.tile([128, 5 * XTF], bf16, name="xat")
        pgg = ctx.enter_context(tc.tile_pool(name="pgg", bufs=1))
        gg = pgg.tile([128, 81 * 128], bf16, name="gg")
        pot = ctx.enter_context(tc.tile_pool(name="pot", bufs=1))
        offt = pot.tile([128, 128 * 32], bf16, name="offt")
        otv = offt[:].rearrange("x (y d) -> x y d", d=32)

        # mask-chain pool opens before the conv pools (LIFO) since
        # masks(0) is emitted while they are still open
        pg = ctx.enter_context(tc.tile_pool(name="pg", bufs=1))

        # ---- offset conv, 4 quarter-rounds ----
        poa_cm = tc.tile_pool(name="poa", bufs=1)
        poa = poa_cm.__enter__()
        pxp_cm = tc.tile_pool(name="pxp", bufs=1)
        pxp = pxp_cm.__enter__()
        xpt = pxp.tile([128, 17408], bf16, name="xpt")
        nc.scalar.dma_start(xpt[:], xpad2[:])
        # x shift-tiles split across both HWDGE queues to land sooner
        for s5 in range(5):
            eng = nc.sync if s5 % 2 == 0 else nc.scalar
            eng.dma_start(
                xat[:, s5 * XTF : (s5 + 1) * XTF],
                xall[:, s5 * XTF : (s5 + 1) * XTF],
            )

        psoff_cm = tc.tile_pool(name="psoff", bufs=2, space="PSUM")
        psoff = psoff_cm.__enter__()

        def conv_round(q):
            rb = q * QF
            stAll = poa.tile([128, QF], bf16, tag="stA", name="stAll")
            stBll = poa.tile([32, QF], bf16, tag="stB", name="stBll")
            offacc = poa.tile([32, QF], bf16, tag="oac", name="offacc")
            nc.gpsimd.memset(offacc[:], 0.0)
            for qq in range(8):
                b0 = qq * 512
                pA = psoff.tile([128, 512], f32, tag="pA", name="pA")
                pB = psoff.tile([32, 512], f32, tag="pB", name="pB")
                for blk in range(3):
                    rhs = xpt[:, rb + b0 + blk * 256 :][:, 0:512]
                    nc.tensor.matmul(
                        pA[:], wAt[:, blk * 128 : blk * 128 + 128], rhs,
                        start=(blk == 0), stop=(blk == 2),
                    )
                for blk in range(3):
                    rhs = xpt[:, rb + b0 + blk * 256 :][:, 0:512]
                    nc.tensor.matmul(
                        pB[0:18, :], wBt[:, blk * 18 : blk * 18 + 18], rhs,
                        start=(blk == 0), stop=(blk == 2),
                    )
                nc.scalar.copy(stAll[:, b0 : b0 + 512], pA[:])
                nc.scalar.copy(stBll[0:18, b0 : b0 + 512], pB[0:18, :])
            # init: kx=2 block
            nc.gpsimd.dma_start(offacc[0:18, :], stAll[64:82, :])
            # 4 shifted accumulates over the quarter
            oav = offacc[:].rearrange("d (y x) -> d y x", x=W)
            sAv = stAll[:].rearrange("d (y x) -> d y x", x=W)
            sBv = stBll[:].rearrange("d (y x) -> d y x", x=W)
            for base, co in ((0, -2), (32, -1), (96, 1), (-1, 2)):
                xs_, xe_ = max(0, -co), min(W, W - co)
                if base < 0:
                    srcw = sBv[0:18, :, xs_ + co : xe_ + co]
                else:
                    srcw = sAv[base : base + 18, :, xs_ + co : xe_ + co]
                nc.gpsimd.dma_start(
                    out=oav[0:18, :, xs_:xe_], in_=srcw, accum_op=Alu.add
                )
            nc.scalar.dma_start_transpose(
                otv[:, q * 32 : q * 32 + 32, :], offacc[:]
            )
            if DEBUG_DUMPS:
                nc.sync.dma_start(dbg_off[:, q * QF : (q + 1) * QF], offacc[:])

        def masks(q):
            # offsets arrive unclamped; the chain clamps to [-1, 1] first
            ovw = mkap(offt[:], q * 32 * 32, [[32, 32], [1, 18]])
            clp = pg.tile([128, 32 * 18], bf16, tag="clp", name="clp")
            mneg = pg.tile([128, 32 * 18], bf16, tag="mneg", name="mneg")
            fr = pg.tile([128, 32 * 18], bf16, tag="fr", name="fr")
            omf = pg.tile([128, 32 * 18], bf16, tag="omf", name="omf")
            g_yd = pg.tile([128, 3 * 32 * 18], bf16, tag="gyd", name="g_yd")
            g_dy = pg.tile([128, 3 * 32 * 18], bf16, tag="gdy", name="g_dy")

            def yd(t):
                return t[:].rearrange("x (y d) -> x y d", d=18)

            def gyd(r):
                return g_yd[:, r * 576 : (r + 1) * 576].rearrange(
                    "x (y d) -> x y d", d=18
                )

            nc.vector.tensor_tensor(  # + bias (broadcast over y)
                out=yd(clp), in0=ovw,
                in1=mkap(offbt[:], 0, [[0, 32], [1, 18]]), op=Alu.add,
            )
            nc.vector.tensor_scalar(
                out=yd(clp), in0=yd(clp), scalar1=1.0, scalar2=-1.0,
                op0=Alu.min, op1=Alu.max,
            )
            nc.vector.tensor_scalar(
                out=yd(mneg), in0=yd(clp), scalar1=0.0, scalar2=None,
                op0=Alu.is_lt,
            )
            nc.vector.tensor_tensor(
                out=yd(fr), in0=yd(clp), in1=yd(mneg), op=Alu.add
            )
            nc.vector.tensor_scalar(
                out=yd(omf), in0=yd(fr), scalar1=-1.0, scalar2=1.0,
                op0=Alu.mult, op1=Alu.add,
            )
            nc.vector.tensor_tensor(  # g0 = mneg * omf
                out=gyd(0), in0=yd(mneg), in1=yd(omf), op=Alu.mult)
            nc.vector.tensor_tensor(  # mneg <- t1 = mneg * fr
                out=yd(mneg), in0=yd(mneg), in1=yd(fr), op=Alu.mult)
            nc.vector.tensor_tensor(  # g2 = fr - t1
                out=gyd(2), in0=yd(fr), in1=yd(mneg), op=Alu.subtract)
            nc.vector.tensor_tensor(  # g1 = t1 + omf
                out=gyd(1), in0=yd(mneg), in1=yd(omf), op=Alu.add)
            nc.vector.tensor_tensor(  # g1 -= g0
                out=gyd(1), in0=gyd(1), in1=gyd(0), op=Alu.subtract)
            # (r, y, d) -> (r, d, y) on Pool (Act is busy with conv copies)
            nc.gpsimd.tensor_copy(
                out=g_dy[:].rearrange("x (r d y) -> x r y d", r=3, d=18),
                in_=g_yd[:].rearrange("x (r y d) -> x r y d", r=3, d=18),
            )
            # gg[x, ti, tj, ry, rx, y-slice] = gy * gx
            for ti in range(3):
                for ry in range(3):
                    o_ap = mkap(
                        gg[:], ti * 3456 + ry * 384 + q * 32,
                        [[1152, 3], [128, 3], [1, 32]],
                    )
                    gy_ap = mkap(
                        g_dy[:], ry * 576 + (6 * ti) * 32,
                        [[64, 3], [0, 3], [1, 32]],
                    )
                    gx_ap = mkap(
                        g_dy[:], (6 * ti + 1) * 32,
                        [[64, 3], [576, 3], [1, 32]],
                    )
                    nc.vector.tensor_tensor(
                        out=o_ap, in0=gy_ap, in1=gx_ap, op=Alu.mult
                    )

        def sampling(q):
            pm = ppm.tile([128, 2048], f32, tag="pm", name="pm")

            def emit_wM(pair, sch2):
                # taps (2*pair, 2*pair+1) contracted together (128 parts)
                for j in range(8):
                    pr = 64 * (j // 4)
                    pc = (j % 4) * 512
                    nc.tensor.matmul(
                        pm[pr : pr + 64, pc : pc + 512],
                        wM2t[:, pair * 64 : (pair + 1) * 64],
                        sch2[:, 4 * j : 4 * j + 4, :].rearrange(
                            "c a x -> c (a x)"
                        ),
                        start=(pair == 0), stop=False,
                    )

            def emit_wM8(sch8):
                for j in range(8):
                    pr = 64 * (j // 4)
                    pc = (j % 4) * 512
                    nc.tensor.matmul(
                        pm[pr : pr + 64, pc : pc + 512],
                        wMt[:, 8 * 64 : 9 * 64],
                        sch8[:, 4 * j : 4 * j + 4, :].rearrange(
                            "c a x -> c (a x)"
                        ),
                        start=False, stop=True,
                    )

            pending = None
            sch2 = None
            for t in range(NT):
                ti, tj = t // 3, t % 3
                spA = psp.tile([128, 1024], f32, tag="spA", name="spA")
                spB = psp.tile([128, 1024], f32, tag="spB", name="spB")

                def product(s, eng, pool):
                    ry, rx = s // 3, s % 3
                    m = t * 9 + s
                    xoff = (tj + rx) * XTF + (ti + ry) + q * 32
                    prod = pool.tile(
                        [128, 2048], bf16, tag="prod", name="prod"
                    )
                    eng.tensor_tensor(
                        out=mkap(prod[:], 0, [[32, 64], [1, 32]]),
                        in0=mkap(xat[:], xoff, [[YP, 64], [1, 32]]),
                        in1=mkap(
                            gg[:], m * 128 + q * 32, [[0, 64], [1, 32]]
                        ),
                        op=Alu.mult,
                    )
                    return prod

                # the last two products are computed on Pool, launched at
                # tap start so they are ready when PE consumes them last
                prod7 = product(7, nc.gpsimd, pp8)
                prod8 = product(8, nc.gpsimd, pp8)
                prods = [product(s, nc.vector, ptmp) for s in range(7)]
                prods += [prod7, prod8]
                for s in range(NT):
                    for c4 in range(4):
                        sph = spA if c4 < 2 else spB
                        pc = (c4 % 2) * 512
                        nc.tensor.matmul(
                            sph[:, pc : pc + 512],
                            idt[:],
                            prods[s][:, c4 * 512 : (c4 + 1) * 512],
                            start=(s == 0), stop=(s == 8),
                        )
                # PSUM->SBUF copy doubles as the (c,y)->(y,c) reorder.
                # Tap pairs share one double-wide ayc tile: even tap fills
                # cols [0:64) of each 128-block, odd tap cols [64:128); one
                # xbar transpose then yields both taps stacked on 128
                # partitions for the paired weight matmul.
                if t % 2 == 0:
                    ayc2 = pfin.tile(
                        [128, 32 * 128], bf16, tag="ayc", name="ayc"
                    )
                # two half-c drain copies: each depends only on its own
                # PSUM half, so the next tap can reuse half A while half B
                # is still draining
                nc.scalar.copy(
                    out=mkap(ayc2[:], 64 * (t % 2), [[128, 32], [1, 32]]),
                    in_=mkap(spA[:], 0, [[1, 32], [32, 32]]),
                )
                nc.scalar.copy(
                    out=mkap(ayc2[:], 64 * (t % 2) + 32, [[128, 32], [1, 32]]),
                    in_=mkap(spB[:], 0, [[1, 32], [32, 32]]),
                )
                if t == 8:
                    sch8full = pfin.tile(
                        [128, 32, 128], bf16, tag="sch", name="sch8"
                    )
                    sch8 = sch8full[0:64, :, :]
                    nc.sync.dma_start_transpose(sch8, ayc2[:])
                    # t=8 pairs with nothing; its half is cols [0:64)
                elif t % 2 == 1:
                    sch2 = pfin.tile(
                        [128, 32, 128], bf16, tag="sch", name="sch"
                    )
                    nc.sync.dma_start_transpose(sch2[:], ayc2[:])
                    new_pair = (t // 2, sch2)
                    if pending is not None:
                        emit_wM(*pending)
                    pending = new_pair
            emit_wM(*pending)
            emit_wM8(sch8)
            ost = post.tile([128, 2048], f32, tag="ost", name="ost")
            nc.scalar.copy(ost[:], pm[:])
            for grp in range(2):
                nc.sync.dma_start(
                    out[:, q * 4096 + grp * 2048 : q * 4096 + (grp + 1) * 2048],
                    ost[64 * grp : 64 * grp + 64, :],
                )

        # emission order: conv q0; masks q0 (unblocks DVE early); conv
        # q1-q3; then sampling q with masks q+1 pipelined behind it.
        conv_round(0)
        masks(0)
        for q in (1, 2, 3):
            conv_round(q)
        psoff_cm.__exit__(None, None, None)
        pxp_cm.__exit__(None, None, None)
        poa_cm.__exit__(None, None, None)
        ptmp = ctx.enter_context(tc.tile_pool(name="ptmp", bufs=6))
        pp8 = ctx.enter_context(tc.tile_pool(name="pp8", bufs=2))
        pfin = ctx.enter_context(tc.tile_pool(name="pfin", bufs=2))
        post = ctx.enter_context(tc.tile_pool(name="post", bufs=1))
        psp = ctx.enter_context(tc.tile_pool(name="psp", bufs=1, space="PSUM"))
        ppm = ctx.enter_context(tc.tile_pool(name="ppm", bufs=1, space="PSUM"))
        for q in range(4):
            sampling(q)
            if q < 3:
                masks(q + 1)

    nc.compile()
    return nc


_NC = None


def _get_nc():
    global _NC
    if _NC is None:
        _NC = _build()
    return _NC


def _pack_inputs(x, weights, offset_w, offset_b):
    # xall: 5 shifted transposed tiles [x, s, (c, 2+y+2)], zeros at borders
    xall = np.zeros((B, 128, 5, C, YP), np.float32)
    for s5 in range(5):
        s = s5 - 2
        xs_, xe_ = max(0, -s), min(W, W - s)
        xall[:, xs_:xe_, s5, :, 2 : 2 + H] = x[:, :, :, xs_ + s : xe_ + s].transpose(
            0, 3, 1, 2
        )
    xall = xall.reshape(B, 128, 5 * C * YP)

    # xpad2: [(c,j), 256 + yx + 768], row (c,j) = flat x[c] shifted j rows
    xpad2 = np.zeros((B, C, 2, 17408), np.float32)
    xf = x.reshape(B, C, HW)
    xpad2[:, :, 0, 256 : 256 + HW] = xf
    xpad2[:, :, 1, 128 : 128 + HW] = xf
    xpad2 = xpad2.reshape(B, 128, 17408)

    # wA2: 3 lhsT blocks [(c,j), (kx<=3 blocks of 32, 18 used)]
    # blk0: ky=(0,1), blk1: ky=(2,3), blk2: ky=4 (j=1 zeroed)
    wA2 = np.zeros((3, C, 2, 128), np.float32)
    wB2 = np.zeros((3, C, 2, 18), np.float32)
    for blk, (ky0, ky1) in enumerate(((0, 1), (2, 3), (4, None))):
        for kx in range(4):
            wA2[blk, :, 0, kx * 32 : kx * 32 + 18] = offset_w[
                :, :, ky0, kx
            ].transpose(1, 0)
            if ky1 is not None:
                wA2[blk, :, 1, kx * 32 : kx * 32 + 18] = offset_w[
                    :, :, ky1, kx
                ].transpose(1, 0)
        wB2[blk, :, 0, :] = offset_w[:, :, ky0, 4].transpose(1, 0)
        if ky1 is not None:
            wB2[blk, :, 1, :] = offset_w[:, :, ky1, 4].transpose(1, 0)
    wA2 = wA2.transpose(1, 2, 0, 3).reshape(128, 3 * 128)
    wB2 = wB2.transpose(1, 2, 0, 3).reshape(128, 3 * 18)

    wMt_ = weights.reshape(C, C, 9).transpose(2, 1, 0)  # [t, c, o]
    wM = np.ascontiguousarray(wMt_.transpose(1, 0, 2)).reshape(C, NT * 64)
    # wM2: pair p stacks taps (2p, 2p+1) on partitions [0:64],[64:128]
    wM2 = np.zeros((2, C, 4, 64), np.float32)
    for p in range(4):
        wM2[0, :, p] = wMt_[2 * p]
        wM2[1, :, p] = wMt_[2 * p + 1]
    wM2 = wM2.reshape(128, 256)

    cast = lambda a: np.ascontiguousarray(a).astype(ml_dtypes.bfloat16)
    id128 = np.eye(128, dtype=np.float32)
    return [
        {
            "xall": cast(xall[b]),
            "xpad2": cast(xpad2[b]),
            "wA2": cast(wA2),
            "wB2": cast(wB2),
            "offbx": cast(np.tile(offset_b[None, :], (128, 1))),
            "wM": cast(wM),
            "wM2": cast(wM2),
            "id128": cast(id128),
        }
        for b in range(B)
    ]


def kernel(x, weights, offset_w, offset_b):
    x = np.asarray(x, dtype=np.float32)
    weights = np.asarray(weights, dtype=np.float32)
    offset_w = np.asarray(offset_w, dtype=np.float32)
    offset_b = np.asarray(offset_b, dtype=np.float32)

    in_maps = _pack_inputs(x, weights, offset_w, offset_b)
    nc = _get_nc()
    # Per-sample sequential execution (8-core shard_map path previously hit
    # an engine hang; single-core runs are the safe path).
    outs = []
    for b in range(B):
        r1 = run_bass_kernel_spmd(nc, [in_maps[b]], [0])
        outs.append(np.asarray(r1.results[0]["out"]))
    return np.stack([o.reshape(C, H, W) for o in outs]).astype(np.float32)
